# revision 15
# baseline (speedup 1.0000x reference)
"""Bahdanau attention Trainium2 Bass kernel.

Problem (fixed shapes):
  decoder_state [32, 1024] f32, encoder_hiddens [32, 2048, 1024] f32,
  Wa_w [1,1024], Wa_b [1], Wb_w [1024,1024], Wb_b [1024], Wc_w [1024,1024], Wc_b [1024]
  out: context [32, 1024] f32

Strategy: data-parallel over batch, 4 batches per core on 8 cores. All
matmuls run as float32r (TF32-like) on the PE. encoder_hiddens is loaded
in natural [s,h] layout and transposed on-chip with PE transpose-mode to
feed the h-contraction matmuls; softmax is computed per 512-wide s-block
(flash style, block max + rescale at batch end) so encoder data is read
exactly once.
"""
import sys

if "/opt/trn_rl_repo" not in sys.path:
    sys.path.insert(0, "/opt/trn_rl_repo")

import numpy as np

import concourse.bass as bass
import concourse.tile as tile
from concourse import bacc, mybir
from concourse import bass_utils
from concourse.masks import make_identity

F32 = mybir.dt.float32
F32R = mybir.dt.float32r

B, S, H, K = 32, 2048, 1024, 1024
NCORES = 8
BLOC = B // NCORES          # batches per core
SBLK = 512                  # s-block (softmax block, PE moving width)
NBLK = S // SBLK            # 4
NST = SBLK // 128           # s-tiles per block: 4
NHT = H // 128              # 8
NKT = K // 128              # 8


def build_kernel():
    nc = bacc.Bacc("TRN2", target_bir_lowering=False)

    enc = nc.dram_tensor("enc", [BLOC, S, H], F32, kind="ExternalInput")
    dec = nc.dram_tensor("dec", [BLOC, H], F32, kind="ExternalInput")
    wa = nc.dram_tensor("wa", [1, K], F32, kind="ExternalInput")
    wb = nc.dram_tensor("wb", [K, H], F32, kind="ExternalInput")
    wbb = nc.dram_tensor("wbb", [1, K], F32, kind="ExternalInput")
    wc = nc.dram_tensor("wc", [K, H], F32, kind="ExternalInput")
    wcb = nc.dram_tensor("wcb", [1, K], F32, kind="ExternalInput")
    y = nc.dram_tensor("y", [BLOC, H], F32, kind="ExternalOutput")

    TT = mybir.ActivationFunctionType.Tanh
    EX = mybir.ActivationFunctionType.Exp
    ADD = mybir.AluOpType.add
    MULT = mybir.AluOpType.mult

    from contextlib import ExitStack
    with tile.TileContext(nc) as tc, ExitStack() as stack:
        consts = stack.enter_context(tc.tile_pool(name="consts", bufs=1))
        identf = consts.tile([128, 128], F32)
        make_identity(nc, identf)
        ident = consts.tile([128, 128], F32R)
        nc.vector.tensor_copy(ident, identf)
        wcT = consts.tile([128, NHT * NKT * 128], F32R)
        waT = consts.tile([128, NKT], F32R)
        bias_kb = consts.tile([128, NKT, BLOC], F32)

        # --- enc prefetch + early transposes (keep PE busy from the start) ---
        enc_p = stack.enter_context(tc.tile_pool(name="enc_nat", bufs=3))
        encT_p = stack.enter_context(tc.tile_pool(name="encT", bufs=15))
        ps_tr = stack.enter_context(tc.tile_pool(name="ps_tr", bufs=2, space="PSUM"))

        def load_enc(b, blk):
            t = enc_p.tile([128, NST, H], F32R, tag="en")
            half = NST // 2
            for hh in range(2):
                nc.gpsimd.dma_start(
                    out=t[:, hh * half:(hh + 1) * half, :],
                    in_=enc[b, blk * SBLK + hh * half * 128:
                            blk * SBLK + (hh + 1) * half * 128, :].rearrange(
                        "(st sp) h -> sp st h", sp=128))
            return t

        def transpose_block(enc_nat):
            encTs = []
            for ht in range(NHT):
                pst = ps_tr.tile([128, SBLK], F32R, tag="tp")
                for st in range(NST):
                    nc.tensor.transpose(pst[:, st * 128:(st + 1) * 128],
                                        enc_nat[:, st, ht * 128:(ht + 1) * 128], ident)
                eT = encT_p.tile([128, SBLK], F32R, tag="eT")
                nc.vector.tensor_copy(eT, pst)
                encTs.append(eT)
            return encTs

        pre = {}
        for bb in ((0, 0), (0, 1)):
            en = load_enc(*bb)
            pre[bb] = (en, transpose_block(en))

        # ---------------- setup: weight transposes + dec_proj ----------------
        with tc.tile_pool(name="setup", bufs=1) as setup, \
             tc.tile_pool(name="setup_ps", bufs=3, space="PSUM") as sps:
            # Wc via HWDGE (f32) — parallel with the SWDGE enc queue
            wc_nat = setup.tile([128, NKT, H], F32, tag="wc_nat")
            nc.sync.dma_start(
                out=wc_nat, in_=wc.rearrange("(kt kp) h -> kp kt h", kp=128))
            # Wb via SWDGE (f32r cast) — queued behind the first two enc blocks
            wb_nat = setup.tile([128, NKT, H], F32R, tag="wb_nat")
            nc.gpsimd.dma_start(
                out=wb_nat, in_=wb.rearrange("(kt kp) h -> kp kt h", kp=128))
            for ktg in range(NKT // 4):
                for ht in range(NHT):
                    ps = sps.tile([128, 512], F32, tag="tp")
                    for kq in range(4):
                        kt = ktg * 4 + kq
                        nc.tensor.transpose(
                            ps[:, kq * 128:(kq + 1) * 128],
                            wc_nat[:, kt, ht * 128:(ht + 1) * 128], identf)
                    nc.vector.tensor_copy(
                        wcT[:, (ht * NKT + ktg * 4) * 128:(ht * NKT + ktg * 4 + 4) * 128], ps)
            # decoder state transposed: decT[h, b] tiles
            dec_nat = setup.tile([BLOC, H], F32)
            nc.sync.dma_start(out=dec_nat, in_=dec[:, :])
            decT = setup.tile([128, NHT, BLOC], F32R)
            for ht in range(NHT):
                ps = sps.tile([128, BLOC], F32, tag="tp")
                nc.tensor.transpose(ps, dec_nat[:, ht * 128:(ht + 1) * 128], identf[0:BLOC, 0:BLOC])
                nc.vector.tensor_copy(decT[:, ht, :], ps)

            # Wa transposed
            wa_nat = setup.tile([1, K], F32)
            nc.sync.dma_start(out=wa_nat, in_=wa[:, :])
            for kt in range(NKT):
                ps = sps.tile([128, 1], F32, tag="tp")
                nc.tensor.transpose(ps, wa_nat[:, kt * 128:(kt + 1) * 128], identf[0:1, 0:1])
                nc.vector.tensor_copy(waT[:, kt:kt + 1], ps)

            # bias rows: Wb_b + Wc_b, transposed to [k,1] segments
            brow = setup.tile([1, K], F32)
            wbb_r = setup.tile([1, K], F32, tag="brin")
            wcb_r = setup.tile([1, K], F32, tag="brin2")
            nc.sync.dma_start(out=wbb_r, in_=wbb[:, :])
            nc.sync.dma_start(out=wcb_r, in_=wcb[:, :])
            nc.vector.tensor_tensor(out=brow, in0=wbb_r, in1=wcb_r, op=ADD)
            bseg = setup.tile([128, NKT], F32)
            for kt in range(NKT):
                ps = sps.tile([128, 1], F32, tag="tp")
                nc.tensor.transpose(ps, brow[:, kt * 128:(kt + 1) * 128], identf[0:1, 0:1])
                nc.vector.tensor_copy(bseg[:, kt:kt + 1], ps)

            # dec_proj[k-tile, b] = sum_h WbT[h,k].T @ decT[h,b]  (+ bias)
            for kt in range(NKT):
                wbT_kt = setup.tile([128, H], F32R, tag="wbT_kt", )
                for htg in range(2):
                    ps = sps.tile([128, 512], F32R, tag="tpb")
                    for hq in range(4):
                        ht = htg * 4 + hq
                        nc.tensor.transpose(
                            ps[:, hq * 128:(hq + 1) * 128],
                            wb_nat[:, kt, ht * 128:(ht + 1) * 128], ident)
                    nc.vector.tensor_copy(wbT_kt[:, htg * 512:(htg + 1) * 512], ps)
                psd = sps.tile([128, BLOC], F32, tag="tp")
                for ht in range(NHT):
                    nc.tensor.matmul(psd, wbT_kt[:, ht * 128:(ht + 1) * 128],
                                     decT[:, ht, :], start=(ht == 0), stop=(ht == NHT - 1))
                bs = bseg[:, kt:kt + 1]
                nc.vector.tensor_tensor(
                    out=bias_kb[:, kt, :], in0=psd,
                    in1=bass.AP(tensor=bs.tensor, offset=bs.offset,
                                ap=[bs.ap[0], [0, BLOC]]),
                    op=ADD)

        # ---------------- main loop ----------------
        e_p = stack.enter_context(tc.tile_pool(name="e", bufs=10))
        row_p = stack.enter_context(tc.tile_pool(name="rows", bufs=3))
        stat_p = stack.enter_context(tc.tile_pool(name="stats", bufs=2))
        cblk_p = stack.enter_context(tc.tile_pool(name="cblk", bufs=2))
        accr_p = stack.enter_context(tc.tile_pool(name="accr", bufs=1))
        ps_e = stack.enter_context(tc.tile_pool(name="ps_e", bufs=2, space="PSUM"))
        ps_s = stack.enter_context(tc.tile_pool(name="ps_s", bufs=1, space="PSUM"))
        ps_c = stack.enter_context(tc.tile_pool(name="ps_c", bufs=2, space="PSUM"))
        ps_w = stack.enter_context(tc.tile_pool(name="ps_w", bufs=1, space="PSUM"))

        def do_context(task):
            # deferred tail of a block: w-transposes + context partial MMs.
            # Runs one block late so PE never waits on the softmax chain.
            blk, enc_nat, wrow, cblks = task
            psw = ps_w.tile([128, NST], F32, tag="wt")
            for st in range(NST):
                nc.tensor.transpose(psw[:, st:st + 1],
                                    wrow[:, st * 128:(st + 1) * 128], identf[0:1, 0:1])
            wT = row_p.tile([128, NST], F32R, tag="wT")
            nc.vector.tensor_copy(wT, psw)
            for hb in range(2):
                psc = ps_c.tile([1, 512], F32, tag="ctx")
                for st in range(NST):
                    nc.tensor.matmul(psc, wT[:, st:st + 1],
                                     enc_nat[:, st, hb * 512:(hb + 1) * 512],
                                     start=(st == 0), stop=(st == NST - 1))
                nc.vector.tensor_copy(cblks[:, blk, hb * 512:(hb + 1) * 512], psc)

        def do_combine(task):
            # batch combine: context = sum_blk C_blk * exp(m_blk - m_g) / Z
            b, mrow, zrow, cblks = task
            negmg = stat_p.tile([1, 1], F32, tag="negmg")
            nc.vector.reduce_max(negmg, mrow, axis=mybir.AxisListType.X, negate=True)
            fb = stat_p.tile([1, NBLK], F32, tag="fb")
            nc.scalar.activation(fb, mrow, EX, bias=negmg)
            zf = stat_p.tile([1, NBLK], F32, tag="zf")
            nc.vector.tensor_tensor(out=zf, in0=zrow, in1=fb, op=MULT)
            z = stat_p.tile([1, 1], F32, tag="z")
            nc.vector.reduce_sum(z, zf, axis=mybir.AxisListType.X)
            rz = stat_p.tile([1, 1], F32, tag="rz")
            nc.vector.reciprocal(rz, z)

            acc = accr_p.tile([1, H], F32, tag="acc")
            nc.vector.tensor_scalar_mul(acc, cblks[:, 0, :], fb[:, 0:1])
            for blk in range(1, NBLK):
                nc.vector.scalar_tensor_tensor(
                    out=acc, in0=cblks[:, blk, :], scalar=fb[:, blk:blk + 1],
                    in1=acc, op0=MULT, op1=ADD)
            ctx_row = accr_p.tile([1, H], F32, tag="ctxr")
            nc.vector.tensor_scalar_mul(ctx_row, acc, rz)
            nc.sync.dma_start(out=y[b:b + 1, :], in_=ctx_row)

        cur = None
        pending_ctx = None
        pending_fin = None
        for b in range(BLOC):
            mrow = stat_p.tile([1, NBLK], F32, tag="mrow")
            zrow = stat_p.tile([1, NBLK], F32, tag="zrow")
            cblks = cblk_p.tile([1, NBLK, H], F32, tag="cb")
            for blk in range(NBLK):
                nxt = (b, blk + 1) if blk + 1 < NBLK else (b + 1, 0)
                if (b, blk) in pre:
                    enc_nat, encTs = pre.pop((b, blk))
                else:
                    enc_nat, encTs = cur
                if nxt[0] < BLOC and nxt not in pre:
                    nxt_en = load_enc(*nxt)
                else:
                    nxt_en = None

                # enc_proj (k-tiles) + tanh -> e
                e_sb = []
                for kt in range(NKT):
                    pse = ps_e.tile([128, SBLK], F32, tag="pe")
                    for ht in range(NHT):
                        nc.tensor.matmul(pse, wcT[:, (ht * NKT + kt) * 128:(ht * NKT + kt + 1) * 128],
                                         encTs[ht], start=(ht == 0), stop=(ht == NHT - 1))
                    et = e_p.tile([128, SBLK], F32R, tag="et")
                    nc.scalar.activation(et, pse, TT, bias=bias_kb[:, kt, b:b + 1])
                    e_sb.append(et)

                # scores row
                pss = ps_s.tile([1, SBLK], F32, tag="sc")
                for kt in range(NKT):
                    nc.tensor.matmul(pss, waT[:, kt:kt + 1], e_sb[kt],
                                     start=(kt == 0), stop=(kt == NKT - 1))
                srow = row_p.tile([1, SBLK], F32, tag="srow")
                nc.vector.tensor_copy(srow, pss)

                # block softmax: m_blk, w = exp(s - m_blk), Z_blk
                negm = row_p.tile([1, 1], F32, tag="negm")
                nc.vector.reduce_max(negm, srow, axis=mybir.AxisListType.X, negate=True)
                wrow = row_p.tile([1, SBLK], F32, tag="wrow")
                nc.scalar.activation(wrow, srow, EX, bias=negm,
                                     accum_out=zrow[:, blk:blk + 1])
                nc.vector.tensor_scalar_mul(mrow[:, blk:blk + 1], negm, -1.0)

                # deferred tail of the previous block, then its batch combine
                if pending_ctx is not None:
                    do_context(pending_ctx)
                    pending_ctx = None
                if pending_fin is not None:
                    do_combine(pending_fin)
                    pending_fin = None
                pending_ctx = (blk, enc_nat, wrow, cblks)
                if blk == NBLK - 1:
                    pending_fin = (b, mrow, zrow, cblks)

                # transpose the next block at the section end (its DMA had a
                # full section to land)
                if nxt_en is not None:
                    cur = (nxt_en, transpose_block(nxt_en))

        do_context(pending_ctx)
        do_combine(pending_fin)

    nc.compile()
    return nc


_NC_CACHE = None


def _get_nc():
    global _NC_CACHE
    if _NC_CACHE is None:
        _NC_CACHE = build_kernel()
    return _NC_CACHE


def kernel(decoder_state, encoder_hiddens, Wa_w, Wa_b, Wb_w, Wb_b, Wc_w, Wc_b,
           **run_kwargs):
    decoder_state = np.ascontiguousarray(decoder_state, dtype=np.float32)
    encoder_hiddens = np.ascontiguousarray(encoder_hiddens, dtype=np.float32)
    nc = _get_nc()
    in_maps = []
    for c in range(NCORES):
        in_maps.append({
            "enc": encoder_hiddens[c * BLOC:(c + 1) * BLOC],
            "dec": decoder_state[c * BLOC:(c + 1) * BLOC],
            "wa": np.ascontiguousarray(Wa_w, dtype=np.float32).reshape(1, K),
            "wb": np.ascontiguousarray(Wb_w, dtype=np.float32),
            "wbb": np.ascontiguousarray(Wb_b, dtype=np.float32).reshape(1, K),
            "wc": np.ascontiguousarray(Wc_w, dtype=np.float32),
            "wcb": np.ascontiguousarray(Wc_b, dtype=np.float32).reshape(1, K),
        })
    res = bass_utils.run_bass_kernel_spmd(
        nc, in_maps, core_ids=list(range(NCORES)), **run_kwargs)
    out = np.concatenate([res.results[c]["y"] for c in range(NCORES)], axis=0)
    # Wa_b shifts every score equally; softmax is invariant to it.
    if run_kwargs:
        return out, res
    return out
